# revision 31
# baseline (speedup 1.0000x reference)
"""Multi-head causal attention (B=4, S=2048, D=1024, H=16) on 8 Trainium2
NeuronCores via Bass/Tile.

Sharding: core c handles batch b = c//2 and head-group g = c%2 (8 heads,
i.e. columns [512g, 512g+512) of Wq/Wk/Wv and rows [512g, 512g+512) of Wo).
Each core computes its 8 heads' attention and a partial output projection
[S, D]; the host sums the two head-group partials per batch and adds bo.

Matmul operands are fp16 (full-rate 1 cycle/row on the PE; fp32 accumulate
in PSUM); softmax runs in fp32 on ACT/DVE. All values are O(100) or less so
fp16 range is safe, and fp16's 10-bit mantissa keeps the end-to-end error
around 5e-4. Layouts keep every matmul at N=512 moving columns:
  qT/kT:  [dk, s]  (projection emitted transposed: lhsT=W chunk, rhs=X^T)
  v:      [s, dk]  interleaved with a ones column per head ([..v_h.., 1])
          so the attention-V matmul also produces the softmax row-sums
  scores: [sk, sq] (transposed; lhsT=kT chunk, rhs=qT) -> exp -> expT
  AV:     av[65, sq] += v_aug^T @ expT  (row 64 = softmax denominators)
  out:    partial[sq, :] = ctx^T.T @ Wo  (ctx^T is exactly the AV output)

Scheduling: the attention m-loop is ACT(exp)-rate-limited, so all serial
chunk-boundary work (reciprocal, denominator broadcast, ctx normalize,
output projection, and the NEXT chunk's q-projection) is deferred into a
task queue pumped one task per (pair, m) iteration — the PE never idles at
chunk boundaries and stays at full p-state.
"""

import os
import sys
import numpy as np

for _p in ("/opt/trn_rl_repo", "/root/.axon_site/_ro/trn_rl_repo"):
    if _p not in sys.path:
        sys.path.append(_p)

B, S_FULL, D, H, DK = 4, 2048, 1024, 16, 64
GD = 512          # dk span per core (8 heads)
P = 128
NPAIR = GD // P   # 4 head-pairs per core
N_CORES = 8
MASK_NEG = -8.0e9  # multiplied by the 0.125 softmax scale inside exp -> -1e9

_BUILD_CACHE = {}


def _build(s_len, causal, zero_bias):
    from contextlib import ExitStack

    import concourse.tile as tile
    from concourse import bacc, mybir

    dt = mybir.dt
    f32, f16, bf16 = dt.float32, dt.float16, dt.bfloat16
    Exp = mybir.ActivationFunctionType.Exp

    S = s_len
    SJ = S // 512     # 512-wide sq chunks
    SM = S // P       # 128-wide sk chunks
    DC = D // P       # contraction chunks for the projections

    nc = bacc.Bacc("TRN2", target_bir_lowering=False, debug=False,
                   num_devices=N_CORES)

    xq = nc.dram_tensor("xq", [D, S], f16, kind="ExternalInput")
    xk = nc.dram_tensor("xk", [D, S], f16, kind="ExternalInput")
    xv = nc.dram_tensor("xv", [D, S], f16, kind="ExternalInput")
    wq = nc.dram_tensor("wq", [D, GD], f16, kind="ExternalInput")
    wk = nc.dram_tensor("wk", [D, GD], f16, kind="ExternalInput")
    wv = nc.dram_tensor("wv", [D, GD], f16, kind="ExternalInput")
    wo = nc.dram_tensor("wo", [GD, D], f16, kind="ExternalInput")
    bq = nc.dram_tensor("bq", [1, GD], f16, kind="ExternalInput")
    bk = nc.dram_tensor("bk", [1, GD], f16, kind="ExternalInput")
    bv = nc.dram_tensor("bv", [1, GD], f16, kind="ExternalInput")
    ones_row = nc.dram_tensor("ones_row", [1, 512], f16, kind="ExternalInput")
    ones_col = nc.dram_tensor("ones_col", [1, P], f16, kind="ExternalInput")
    ones_vcol = nc.dram_tensor("ones_vcol", [P, 8], f16, kind="ExternalInput")
    # selh[:, 64r:64r+64] is row-r-one-hot: selects head r's reciprocal row
    # and broadcasts it over 64 partitions in one K=8 matmul
    selh = nc.dram_tensor("selh", [8, 8 * 64], f16, kind="ExternalInput")
    # oneh8[0, 8r:8r+8] is the one-hot row e_r: routes head r's softmax
    # denominator row into partition r of the gathered [8, 512] psum tile
    oneh8 = nc.dram_tensor("oneh8", [1, 64], f16, kind="ExternalInput")
    if causal:
        # 4 canonical diagonal-band blocks: block d, entry [p, c] masked
        # when p + 128*d > c  (value MASK_NEG, else 0)
        maskd = nc.dram_tensor("maskd", [4 * P, 512], bf16, kind="ExternalInput")
    else:
        # full transposed mask [sk, sq] * MASK_NEG
        maskt = nc.dram_tensor("maskt", [S, S], bf16, kind="ExternalInput")
    out = nc.dram_tensor("out", [S, D], f32, kind="ExternalOutput")

    with tile.TileContext(nc) as tc, ExitStack() as ctx0:
        persist = ctx0.enter_context(tc.tile_pool(name="persist", bufs=1))

        kTt = [persist.tile([P, S], f16, tag=f"kT{p}", name=f"kT{p}")
               for p in range(NPAIR)]
        vt = [persist.tile([P, 8 * 65], f16, tag=f"v{m}", name=f"v{m}")
              for m in range(SM)]
        ctxt = [persist.tile([P, S], f16, tag=f"ctx{p}", name=f"ctx{p}")
                for p in range(NPAIR)]
        # wq persists: the q projection for sq chunk j+1 is emitted as
        # filler inside chunk j's attention loop. One [128, DC*512] tile,
        # dc-major, filled by a single rearranged-AP DMA
        wqt_all = persist.tile([P, DC * GD], f16, tag="wq", name="wq")

        def wq_sl(dc, i):
            return wqt_all[:, dc * GD + i * P:dc * GD + (i + 1) * P]
        wot = [[persist.tile([P, 512], f16, tag=f"wo{p}_{h}", name=f"wo{p}_{h}")
                for h in range(D // 512)] for p in range(NPAIR)]
        if not zero_bias:
            ones_row_t = persist.tile([1, 512], f16, tag="ones_row")
            ones_col_t = persist.tile([1, P], f16, tag="ones_col")
            bq_t = persist.tile([1, GD], f16, tag="bq")
            bk_t = persist.tile([1, GD], f16, tag="bk")
            bv_t = persist.tile([1, GD], f16, tag="bv")
            nc.gpsimd.dma_start(ones_row_t[:], ones_row.ap())
            nc.gpsimd.dma_start(ones_col_t[:], ones_col.ap())
            nc.gpsimd.dma_start(bq_t[:], bq.ap())
            nc.gpsimd.dma_start(bk_t[:], bk.ap())
            nc.gpsimd.dma_start(bv_t[:], bv.ap())
        else:
            ones_row_t = ones_col_t = bq_t = bk_t = bv_t = None
        selh_t = persist.tile([8, 8 * 64], f16, tag="selh")
        # staged at partition 64 so its base matches asb[64:65] in the
        # denominator-gather matmul (lhsT/rhs bases must agree)
        oneh8_t = persist.tile([65, 64], f16, tag="oneh8")
        nc.gpsimd.dma_start(selh_t[:], selh.ap())
        nc.gpsimd.dma_start(oneh8_t[64:65, :], oneh8.ap())
        if causal:
            # one [128,128] block covers every diagonal mixed window:
            # within the window the pattern is always "masked iff p > c"
            mask128_t = persist.tile([P, P], bf16, tag="mask128")
            nc.gpsimd.dma_start(mask128_t[:], maskd.ap()[0:P, 0:P])

        # q-projection staging: per-chunk X^T column slices and per-chunk
        # qT output tiles (chunk j's qT is produced during chunk j-1)
        qpool = ctx0.enter_context(tc.tile_pool(name="qp", bufs=8))
        xqpool = ctx0.enter_context(tc.tile_pool(name="xqp", bufs=2))
        xqc = {}
        qTc = {}

        def load_xq_chunk(j, eng=None):
            # one DMA per chunk: dst is [128, (dc, 512)] dc-major, src AP
            # rearranged so row dc*128+p lands on partition p
            if j >= SJ:
                return
            t = xqpool.tile([P, DC * 512], f16, tag="xqc", name="xqc")
            (eng or nc.gpsimd).dma_start(
                t[:].rearrange("p (dc c) -> p dc c", dc=DC),
                xq.ap()[:, j * 512:(j + 1) * 512].rearrange(
                    "(dc p) c -> p dc c", p=P))
            xqc[j] = t

        # the first matmul group needs wq + xq chunk 0 — those ride the two
        # fast rings as single big DMAs; everything later-needed (xq chunk
        # 1, wo) rides the gpsimd ring
        nc.sync.dma_start(
            wqt_all[:].rearrange("p (dc c) -> p dc c", dc=DC),
            wq.ap().rearrange("(dc p) c -> p dc c", p=P))
        load_xq_chunk(0, eng=nc.scalar)
        load_xq_chunk(1)
        for p in range(NPAIR):
            for h in range(D // 512):
                nc.gpsimd.dma_start(
                    wot[p][h][:],
                    wo.ap()[p * P:(p + 1) * P, h * 512:(h + 1) * 512])

        # attention psum pools are created after phase 1 releases ps1's
        # banks (PSUM pools are statically allocated; 8 banks total)
        scps = avps = None
        expp = ctx0.enter_context(tc.tile_pool(name="expp", bufs=6))
        avsb = ctx0.enter_context(tc.tile_pool(name="avsb", bufs=14))
        rcp = ctx0.enter_context(tc.tile_pool(name="rcp", bufs=2))
        osb = ctx0.enter_context(tc.tile_pool(name="osb", bufs=2))
        if not causal:
            mpool = ctx0.enter_context(tc.tile_pool(name="mp", bufs=SM + 2))

        def emit_qproj(j, i):
            # q projection for pair i of sq chunk j -> qTc[(j, i)]
            ps = avps.tile([P, 512], f32, tag="av", name="qps")
            for dc in range(DC):
                nc.tensor.matmul(
                    ps[:], wq_sl(dc, i),
                    xqc[j][:, dc * 512:(dc + 1) * 512],
                    start=(dc == 0), stop=(zero_bias and dc == DC - 1))
            if not zero_bias:
                nc.tensor.matmul(ps[:], bq_t[0:1, i * P:(i + 1) * P],
                                 ones_row_t[:], start=False, stop=True)
            t = qpool.tile([P, 512], f16, tag="qTc", name="qTc")
            nc.vector.tensor_copy(t[:], ps[:])
            qTc[(j, i)] = t

        def emit_outproj_si(si):
            # output projection for one 128-row block of sq
            ot = osb.tile([P, D], f32, tag="ot", name="ot")
            for h in range(D // 512):
                ps = avps.tile([P, 512], f32, tag="av", name="ps3")
                for p in range(NPAIR):
                    nc.tensor.matmul(
                        ps[:],
                        ctxt[p][:, si * P:(si + 1) * P],
                        wot[p][h][:],
                        start=(p == 0), stop=(p == NPAIR - 1))
                nc.vector.tensor_copy(ot[:, h * 512:(h + 1) * 512], ps[:])
            nc.sync.dma_start(out.ap()[si * P:(si + 1) * P, :], ot[:])

        # k/v inputs for columns 1024: persist into chunk 0, where the
        # second halves of the k/v projections run as deferred PE filler
        # (phase 1 is DMA-bandwidth-bound; chunks 0-1 are compute-bound
        # with an idle DMA, so the later-needed halves load there).
        # All staged X/W tensors are single [128, (dc, cols)] dc-major
        # tiles filled by one rearranged-AP DMA each.
        vbpool = ctx0.enter_context(tc.tile_pool(name="vb", bufs=1))
        vwpool = ctx0.enter_context(tc.tile_pool(name="vw", bufs=1))
        kbpool = ctx0.enter_context(tc.tile_pool(name="kb", bufs=1))
        kwpool = ctx0.enter_context(tc.tile_pool(name="kw", bufs=1))

        def _ld(eng, tile_ap, src_ap, cols):
            eng.dma_start(
                tile_ap.rearrange("p (dc c) -> p dc c", dc=DC),
                src_ap.rearrange("(dc p) c -> p dc c", p=P))

        # ---------------- phase 1: k/v projections (+ q chunk 0) ---------
        kwt_all = kwpool.tile([P, DC * GD], f16, tag="kw", name="kw")
        vwt_all = vwpool.tile([P, DC * GD], f16, tag="vw", name="vw")
        kxB = kbpool.tile([P, DC * 1024], f16, tag="kxb", name="kxb")
        vxB = vbpool.tile([P, DC * 1024], f16, tag="xb", name="xb")
        with ExitStack() as ctx1:
            xpool = ctx1.enter_context(tc.tile_pool(name="xt", bufs=1))
            xapool = ctx1.enter_context(tc.tile_pool(name="xa", bufs=1))
            ps1 = ctx1.enter_context(tc.tile_pool(name="ps1", bufs=3, space="PSUM"))

            # k on the sync ring, v on the scalar ring — only borrowed this
            # early in phase 1; idle again well before the first exp lands
            # on ACT. The later-needed B halves trail on the gpsimd ring.
            _ld(nc.sync, kwt_all[:], wk.ap(), GD)
            kxA = xpool.tile([P, DC * 1024], f16, tag="xt", name="xt")
            _ld(nc.sync, kxA[:], xk.ap()[:, 0:1024], 1024)
            _ld(nc.scalar, vwt_all[:], wv.ap(), GD)
            vxA = xapool.tile([P, DC * 1024], f16, tag="xa", name="xa")
            _ld(nc.scalar, vxA[:], xv.ap()[:, 0:1024], 1024)
            _ld(nc.gpsimd, kxB[:], xk.ap()[:, 1024:2048], 1024)
            _ld(nc.gpsimd, vxB[:], xv.ap()[:, 1024:2048], 1024)

            # q chunk 0 first: unblocks chunk-0 scores as soon as k pair 0
            # lands
            for i in range(NPAIR):
                ps = ps1.tile([P, 512], f32, tag="ps1", name="ps1")
                for dc in range(DC):
                    nc.tensor.matmul(
                        ps[:], wq_sl(dc, i),
                        xqc[0][:, dc * 512:(dc + 1) * 512],
                        start=(dc == 0), stop=(zero_bias and dc == DC - 1))
                if not zero_bias:
                    nc.tensor.matmul(ps[:], bq_t[0:1, i * P:(i + 1) * P],
                                     ones_row_t[:], start=False, stop=True)
                t = qpool.tile([P, 512], f16, tag="qTc", name="qTc")
                nc.vector.tensor_copy(t[:], ps[:])
                qTc[(0, i)] = t

            # k projection for sj 0-1 only (sj 2-3 are deferred into the
            # chunk 0/1 task queues; chunk j only reads kT columns < 512j+512)
            for sj in range(2):
                kx = kxA
                s0 = (sj % 2) * 512
                for i in range(NPAIR):
                    ps = ps1.tile([P, 512], f32, tag="ps1", name="ps1")
                    for dc in range(DC):
                        nc.tensor.matmul(
                            ps[:],
                            kwt_all[:, dc * GD + i * P:dc * GD + (i + 1) * P],
                            kx[:, dc * 1024 + s0:dc * 1024 + s0 + 512],
                            start=(dc == 0),
                            stop=(zero_bias and dc == DC - 1))
                    if not zero_bias:
                        nc.tensor.matmul(
                            ps[:], bk_t[0:1, i * P:(i + 1) * P],
                            ones_row_t[:], start=False, stop=True)
                    nc.vector.tensor_copy(
                        kTt[i][:, sj * 512:(sj + 1) * 512], ps[:])

            # v projection (first half; si 8-15 deferred into chunk 0),
            # natural layout [s, dk] with interleaved ones col
            for si in range(SM // 2):
                ps = ps1.tile([P, 512], f32, tag="ps1", name="ps1")
                for dc in range(DC):
                    nc.tensor.matmul(
                        ps[:],
                        vxA[:, dc * 1024 + si * P:dc * 1024 + (si + 1) * P],
                        vwt_all[:, dc * GD:(dc + 1) * GD],
                        start=(dc == 0),
                        stop=(zero_bias and dc == DC - 1))
                if not zero_bias:
                    nc.tensor.matmul(ps[:], ones_col_t[:], bv_t[:],
                                     start=False, stop=True)
                v3 = vt[si][:].rearrange("p (h c) -> p h c", h=8)
                nc.vector.tensor_copy(
                    v3[:, :, 0:64],
                    ps[:].rearrange("p (h c) -> p h c", h=8))
                nc.vector.memset(v3[:, :, 64:65], 1.0)

        # ---------------- phase 2: attention with deferred fillers -------
        scps = ctx0.enter_context(tc.tile_pool(name="scps", bufs=2, space="PSUM"))
        avps = ctx0.enter_context(tc.tile_pool(name="avps", bufs=3, space="PSUM"))
        fill = []

        def pump():
            while fill:
                t = fill.pop(0)
                if t is None:
                    return  # delay slot
                t()
                return

        def mk_norm(p, e, jj, asb, rv):
            def t():
                r = 2 * p + e
                bc = avps.tile([65, 512], f32, tag="av", name="bc")
                nc.tensor.matmul(bc[0:64, :],
                                 selh_t[:, 64 * r:64 * r + 64],
                                 rv[:], start=True, stop=True)
                nc.vector.tensor_mul(
                    ctxt[p][64 * e:64 * e + 64, jj * 512:(jj + 1) * 512],
                    asb[0:64, :], bc[0:64, :])
            return t

        def mk_vproj(si):
            def t():
                s0 = (si - SM // 2) * P
                ps = avps.tile([P, 512], f32, tag="av", name="vps")
                for dc in range(DC):
                    nc.tensor.matmul(
                        ps[:],
                        vxB[:, dc * 1024 + s0:dc * 1024 + s0 + P],
                        vwt_all[:, dc * GD:(dc + 1) * GD],
                        start=(dc == 0),
                        stop=(zero_bias and dc == DC - 1))
                if not zero_bias:
                    nc.tensor.matmul(ps[:], ones_col_t[:], bv_t[:],
                                     start=False, stop=True)
                v3 = vt[si][:].rearrange("p (h c) -> p h c", h=8)
                nc.vector.tensor_copy(
                    v3[:, :, 0:64],
                    ps[:].rearrange("p (h c) -> p h c", h=8))
                nc.vector.memset(v3[:, :, 64:65], 1.0)
            return t

        def mk_kproj(sj, i):
            def t():
                s0 = (sj - 2) * 512
                ps = avps.tile([P, 512], f32, tag="av", name="kps")
                for dc in range(DC):
                    nc.tensor.matmul(
                        ps[:],
                        kwt_all[:, dc * GD + i * P:dc * GD + (i + 1) * P],
                        kxB[:, dc * 1024 + s0:dc * 1024 + s0 + 512],
                        start=(dc == 0),
                        stop=(zero_bias and dc == DC - 1))
                if not zero_bias:
                    nc.tensor.matmul(
                        ps[:], bk_t[0:1, i * P:(i + 1) * P],
                        ones_row_t[:], start=False, stop=True)
                nc.vector.tensor_copy(
                    kTt[i][:, sj * 512:(sj + 1) * 512], ps[:])
            return t

        # second halves of the k/v projections fill chunk 0-1 PE slack
        # (nothing else is deferred yet there); k sj=2 and v si 8-11 are
        # first needed by chunk 2's m-loop, k sj=3 by chunk 3's
        fill.extend(mk_kproj(2, i) for i in range(NPAIR))
        fill.extend(mk_vproj(si) for si in range(SM // 2, SM))
        fill.extend(mk_kproj(3, i) for i in range(NPAIR))

        for j in range(SJ):
            n_m = 4 * (j + 1) if causal else SM
            # stage the x columns for the q projection two chunks ahead
            load_xq_chunk(j + 2)
            if not causal:
                mt = []
                for m in range(SM):
                    t = mpool.tile([P, 512], bf16, tag="mt", name="mt")
                    nc.sync.dma_start(
                        t[:], maskt.ap()[m * P:(m + 1) * P,
                                         j * 512:(j + 1) * 512])
                    mt.append(t)
            den = avps.tile([8, 512], f32, tag="den", name="den", bufs=1)
            asb_all = {}
            for p in range(NPAIR):
                av = [avps.tile([65, 512], f32, tag="av", name="av")
                      for _ in range(2)]
                pend = []  # (m, exp_tile, c0) awaiting their AV matmuls
                for m in range(n_m):
                    pump()
                    # causal diagonal block d: columns [0, 128d) of this
                    # sq chunk are fully masked -> compute only the
                    # suffix [c0, 512) in scores/exp/AV; the mixed
                    # 128-col window gets the shared p>c mask block
                    d = m - 4 * j if causal else -1
                    c0 = 128 * d if d > 0 else 0
                    nv = 512 - c0
                    sc = scps.tile([P, 1024], f32, tag="sc", name="sc")
                    for e in range(2):
                        nc.tensor.matmul(
                            sc[:, e * 512 + c0:(e + 1) * 512],
                            kTt[p][64 * e:64 * e + 64, m * P:(m + 1) * P],
                            qTc[(j, p)][64 * e:64 * e + 64, c0:512],
                            start=True, stop=True)
                    sc3 = sc[:].rearrange("p (e c) -> p e c", e=2)
                    if causal:
                        if d >= 0:
                            nc.vector.tensor_add(
                                sc3[:, :, c0:c0 + P], sc3[:, :, c0:c0 + P],
                                mask128_t[:][:, None, :].broadcast_to(
                                    [P, 2, P]))
                    else:
                        nc.vector.tensor_add(
                            sc3, sc3,
                            mt[m][:][:, None, :].broadcast_to([P, 2, 512]))
                    ex = expp.tile([P, 1024], f16, tag="ex", name="ex")
                    ex3 = ex[:].rearrange("p (e c) -> p e c", e=2)
                    nc.scalar.activation(ex3[:, :, c0:512],
                                         sc3[:, :, c0:512], Exp, scale=0.125)
                    pend.append((m, ex, c0))
                    if len(pend) > 3:
                        pm, pex, pc0 = pend.pop(0)
                        for e in range(2):
                            nc.tensor.matmul(
                                av[e][:, pc0:512],
                                vt[pm][:, 65 * (2 * p + e):65 * (2 * p + e) + 65],
                                pex[:, e * 512 + pc0:(e + 1) * 512],
                                start=(pm == 0), stop=(pm == n_m - 1))
                for pm, pex, pc0 in pend:
                    # the tail exps lag the PE by a few ACT latencies —
                    # pumped filler keeps the PE busy while they drain
                    pump()
                    for e in range(2):
                        nc.tensor.matmul(
                            av[e][:, pc0:512],
                            vt[pm][:, 65 * (2 * p + e):65 * (2 * p + e) + 65],
                            pex[:, e * 512 + pc0:(e + 1) * 512],
                            start=(pm == 0), stop=(pm == n_m - 1))
                pump()
                # stage av in SBUF; route its denominator row (base
                # partition 64, which matmul rhs allows) into partition
                # 2p+e of the shared den psum tile via a one-hot K=1 MM
                for e in range(2):
                    r = 2 * p + e
                    asb = avsb.tile([65, 512], f16, tag="asb", name="asb")
                    nc.vector.tensor_copy(asb[:], av[e][:])
                    nc.tensor.matmul(den[:], oneh8_t[64:65, 8 * r:8 * r + 8],
                                     asb[64:65, :],
                                     start=(r == 0), stop=(r == 7))
                    asb_all[(p, e)] = asb

            # the batched reciprocal launches on DVE right away (it has no
            # PE dependents until the deferred normalize tasks run), and the
            # NEXT chunk's q projection runs here as boundary filler — the
            # av psum slots are free between chunks, so it doesn't contend
            # with the m-loop's rotating slot
            rv = rcp.tile([8, 512], f16, tag="rinv", name="rinv")
            with nc.allow_low_precision(
                    reason="softmax denominators are O(1..3e4); fp16 "
                           "reciprocal keeps ~5e-4 rel err"):
                nc.vector.reciprocal(rv[:], den[:])
            if j + 1 < SJ:
                for i in range(NPAIR):
                    emit_qproj(j + 1, i)
            # normalize + outproj are deferred into the next chunk's m-loop,
            # behind delay slots so the PE's in-order stream never reaches a
            # reciprocal-dependent instruction early
            nxt = [None] * 2
            for p in range(NPAIR):
                nxt.append(mk_norm(p, 0, j, asb_all[(p, 0)], rv))
                nxt.append(mk_norm(p, 1, j, asb_all[(p, 1)], rv))
            for si in range(4 * j, 4 * j + 4):
                nxt.append(lambda s=si: emit_outproj_si(s))
            fill.extend(nxt)
        while fill:
            pump()

    nc.compile()
    return nc


def _get_nc(s_len, causal, zero_bias):
    key = (s_len, causal, zero_bias)
    if key not in _BUILD_CACHE:
        _BUILD_CACHE[key] = _build(s_len, causal, zero_bias)
    return _BUILD_CACHE[key]


def kernel(query, key, value, mask, Wq, bq, Wk, bk, Wv, bv, Wo, bo):
    import ml_dtypes
    from concourse.bass_utils import run_bass_kernel_spmd

    query = np.asarray(query, dtype=np.float32)
    key = np.asarray(key, dtype=np.float32)
    value = np.asarray(value, dtype=np.float32)
    mask = np.asarray(mask, dtype=np.float32)
    Wq, Wk, Wv, Wo = (np.asarray(w, dtype=np.float32) for w in (Wq, Wk, Wv, Wo))
    bq, bk, bv, bo = (np.asarray(b, dtype=np.float32) for b in (bq, bk, bv, bo))

    b_sz, s_len, d = query.shape
    m2 = mask.reshape(s_len, s_len)
    causal = bool(
        np.array_equal(m2, np.triu(np.ones((s_len, s_len), np.float32), k=1)))

    zero_bias = not (bq.any() or bk.any() or bv.any())
    nc = _get_nc(s_len, causal, zero_bias)

    f16 = np.float16
    ones_row = np.ones((1, 512), f16)
    ones_col = np.ones((1, P), f16)
    ones_vcol = np.ones((P, 8), f16)
    selh = np.zeros((8, 8 * 64), f16)
    for r in range(8):
        selh[r, 64 * r:64 * r + 64] = 1.0
    oneh8 = np.zeros((1, 64), f16)
    oneh8[0, 9 * np.arange(8)] = 1.0
    if causal:
        # maskd[d][p, c] = MASK_NEG where p + 128*d > c
        pp = np.arange(P)[:, None]
        cc = np.arange(512)[None, :]
        maskd = np.concatenate(
            [np.where(pp + P * dd > cc, MASK_NEG, 0.0) for dd in range(4)],
            axis=0).astype(ml_dtypes.bfloat16)
    else:
        maskt = (m2.T * MASK_NEG).astype(ml_dtypes.bfloat16)

    in_maps = []
    for c in range(N_CORES):
        b = c // 2
        g = c % 2
        cols = slice(GD * g, GD * g + GD)
        im = {
            "xq": np.ascontiguousarray(query[b].T).astype(f16),
            "xk": np.ascontiguousarray(key[b].T).astype(f16),
            "xv": np.ascontiguousarray(value[b].T).astype(f16),
            "wq": np.ascontiguousarray(Wq[:, cols]).astype(f16),
            "wk": np.ascontiguousarray(Wk[:, cols]).astype(f16),
            "wv": np.ascontiguousarray(Wv[:, cols]).astype(f16),
            "wo": np.ascontiguousarray(Wo[cols, :]).astype(f16),
            "bq": bq[cols].reshape(1, GD).astype(f16),
            "bk": bk[cols].reshape(1, GD).astype(f16),
            "bv": bv[cols].reshape(1, GD).astype(f16),
            "ones_row": ones_row,
            "ones_col": ones_col,
            "ones_vcol": ones_vcol,
            "selh": selh,
            "oneh8": oneh8,
        }
        if causal:
            im["maskd"] = maskd
        else:
            im["maskt"] = maskt
        in_maps.append(im)

    res = run_bass_kernel_spmd(nc, in_maps, list(range(N_CORES)))

    out = np.empty((b_sz, s_len, d), np.float32)
    for b in range(b_sz):
        out[b] = res.results[2 * b]["out"] + res.results[2 * b + 1]["out"] + bo
    return out


# revision 34
# speedup vs baseline: 1.1418x; 1.1418x over previous
"""Multi-head causal attention (B=4, S=2048, D=1024, H=16) on 8 Trainium2
NeuronCores via Bass/Tile.

Sharding: core c handles batch b = c//2 and head-group g = c%2 (8 heads,
i.e. columns [512g, 512g+512) of Wq/Wk/Wv and rows [512g, 512g+512) of Wo).
Each core computes its 8 heads' attention and a partial output projection
[S, D]; the host sums the two head-group partials per batch and adds bo.

Matmul operands are fp16 (full-rate 1 cycle/row on the PE; fp32 accumulate
in PSUM); softmax runs in fp32 on ACT/DVE. All values are O(100) or less so
fp16 range is safe, and fp16's 10-bit mantissa keeps the end-to-end error
around 5e-4. Layouts keep every matmul at N=512 moving columns:
  qT/kT:  [dk, s]  (projection emitted transposed: lhsT=W chunk, rhs=X^T)
  v:      [s, dk]  interleaved with a ones column per head ([..v_h.., 1])
          so the attention-V matmul also produces the softmax row-sums
  scores: [sk, sq] (transposed; lhsT=kT chunk, rhs=qT) -> exp -> expT
  AV:     av[65, sq] += v_aug^T @ expT  (row 64 = softmax denominators)
  out:    partial[sq, :] = ctx^T.T @ Wo  (ctx^T is exactly the AV output)

Scheduling: the attention m-loop is ACT(exp)-rate-limited, so all serial
chunk-boundary work (reciprocal, denominator broadcast, ctx normalize,
output projection, and the NEXT chunk's q-projection) is deferred into a
task queue pumped one task per (pair, m) iteration — the PE never idles at
chunk boundaries and stays at full p-state.
"""

import os
import sys
import numpy as np

for _p in ("/opt/trn_rl_repo", "/root/.axon_site/_ro/trn_rl_repo"):
    if _p not in sys.path:
        sys.path.append(_p)

B, S_FULL, D, H, DK = 4, 2048, 1024, 16, 64
GD = 512          # dk span per core (8 heads)
P = 128
NPAIR = GD // P   # 4 head-pairs per core
N_CORES = 8
MASK_NEG = -8.0e9  # multiplied by the 0.125 softmax scale inside exp -> -1e9

_BUILD_CACHE = {}


def _build(s_len, causal, zero_bias):
    from contextlib import ExitStack

    import concourse.tile as tile
    from concourse import bacc, mybir

    dt = mybir.dt
    f32, f16, bf16 = dt.float32, dt.float16, dt.bfloat16
    Exp = mybir.ActivationFunctionType.Exp

    S = s_len
    SJ = S // 512     # 512-wide sq chunks
    SM = S // P       # 128-wide sk chunks
    DC = D // P       # contraction chunks for the projections

    nc = bacc.Bacc("TRN2", target_bir_lowering=False, debug=False,
                   num_devices=N_CORES)

    xq = nc.dram_tensor("xq", [D, S], f16, kind="ExternalInput")
    xk = nc.dram_tensor("xk", [D, S], f16, kind="ExternalInput")
    xv = nc.dram_tensor("xv", [D, S], f16, kind="ExternalInput")
    wq = nc.dram_tensor("wq", [D, GD], f16, kind="ExternalInput")
    wk = nc.dram_tensor("wk", [D, GD], f16, kind="ExternalInput")
    wv = nc.dram_tensor("wv", [D, GD], f16, kind="ExternalInput")
    wo = nc.dram_tensor("wo", [GD, D], f16, kind="ExternalInput")
    bq = nc.dram_tensor("bq", [1, GD], f16, kind="ExternalInput")
    bk = nc.dram_tensor("bk", [1, GD], f16, kind="ExternalInput")
    bv = nc.dram_tensor("bv", [1, GD], f16, kind="ExternalInput")
    ones_row = nc.dram_tensor("ones_row", [1, 512], f16, kind="ExternalInput")
    ones_col = nc.dram_tensor("ones_col", [1, P], f16, kind="ExternalInput")
    ones_vcol = nc.dram_tensor("ones_vcol", [P, 8], f16, kind="ExternalInput")
    # selh[:, 64r:64r+64] is row-r-one-hot: selects head r's reciprocal row
    # and broadcasts it over 64 partitions in one K=8 matmul
    selh = nc.dram_tensor("selh", [8, 8 * 64], f16, kind="ExternalInput")
    # oneh8[0, 8r:8r+8] is the one-hot row e_r: routes head r's softmax
    # denominator row into partition r of the gathered [8, 512] psum tile
    oneh8 = nc.dram_tensor("oneh8", [1, 64], f16, kind="ExternalInput")
    if causal:
        # 4 canonical diagonal-band blocks: block d, entry [p, c] masked
        # when p + 128*d > c  (value MASK_NEG, else 0)
        maskd = nc.dram_tensor("maskd", [4 * P, 512], bf16, kind="ExternalInput")
    else:
        # full transposed mask [sk, sq] * MASK_NEG
        maskt = nc.dram_tensor("maskt", [S, S], bf16, kind="ExternalInput")
    out = nc.dram_tensor("out", [S, D], f32, kind="ExternalOutput")

    with tile.TileContext(nc) as tc, ExitStack() as ctx0:
        persist = ctx0.enter_context(tc.tile_pool(name="persist", bufs=1))

        kTt = [persist.tile([P, S], f16, tag=f"kT{p}", name=f"kT{p}")
               for p in range(NPAIR)]
        vt = [persist.tile([P, 8 * 65], f16, tag=f"v{m}", name=f"v{m}")
              for m in range(SM)]
        ctxt = [persist.tile([P, S], f16, tag=f"ctx{p}", name=f"ctx{p}")
                for p in range(NPAIR)]
        # wq persists: the q projection for sq chunk j+1 is emitted as
        # filler inside chunk j's attention loop. One [128, DC*512] tile,
        # dc-major, filled by a single rearranged-AP DMA
        wqt_all = persist.tile([P, DC * GD], f16, tag="wq", name="wq")

        def wq_sl(dc, i):
            return wqt_all[:, dc * GD + i * P:dc * GD + (i + 1) * P]
        wot = [[persist.tile([P, 512], f16, tag=f"wo{p}_{h}", name=f"wo{p}_{h}")
                for h in range(D // 512)] for p in range(NPAIR)]
        if not zero_bias:
            ones_row_t = persist.tile([1, 512], f16, tag="ones_row")
            ones_col_t = persist.tile([1, P], f16, tag="ones_col")
            bq_t = persist.tile([1, GD], f16, tag="bq")
            bk_t = persist.tile([1, GD], f16, tag="bk")
            bv_t = persist.tile([1, GD], f16, tag="bv")
            nc.gpsimd.dma_start(ones_row_t[:], ones_row.ap())
            nc.gpsimd.dma_start(ones_col_t[:], ones_col.ap())
            nc.gpsimd.dma_start(bq_t[:], bq.ap())
            nc.gpsimd.dma_start(bk_t[:], bk.ap())
            nc.gpsimd.dma_start(bv_t[:], bv.ap())
        else:
            ones_row_t = ones_col_t = bq_t = bk_t = bv_t = None
        selh_t = persist.tile([8, 8 * 64], f16, tag="selh")
        # staged at partition 64 so its base matches asb[64:65] in the
        # denominator-gather matmul (lhsT/rhs bases must agree)
        oneh8_t = persist.tile([65, 64], f16, tag="oneh8")
        nc.gpsimd.dma_start(selh_t[:], selh.ap())
        nc.gpsimd.dma_start(oneh8_t[64:65, :], oneh8.ap())
        if causal:
            # one [128,128] block covers every diagonal mixed window:
            # within the window the pattern is always "masked iff p > c"
            mask128_t = persist.tile([P, P], bf16, tag="mask128")
            nc.gpsimd.dma_start(mask128_t[:], maskd.ap()[0:P, 0:P])

        # q-projection staging: per-chunk X^T column slices and per-chunk
        # qT output tiles (chunk j's qT is produced during chunk j-1)
        qpool = ctx0.enter_context(tc.tile_pool(name="qp", bufs=8))
        xqpool = ctx0.enter_context(tc.tile_pool(name="xqp", bufs=2))
        xqc = {}
        qTc = {}

        def load_xq_chunk(j, eng=None):
            # one DMA per chunk: dst is [128, (dc, 512)] dc-major, src AP
            # rearranged so row dc*128+p lands on partition p
            if j >= SJ:
                return
            t = xqpool.tile([P, DC * 512], f16, tag="xqc", name="xqc")
            (eng or nc.gpsimd).dma_start(
                t[:].rearrange("p (dc c) -> p dc c", dc=DC),
                xq.ap()[:, j * 512:(j + 1) * 512].rearrange(
                    "(dc p) c -> p dc c", p=P))
            xqc[j] = t

        # k/v inputs for columns 1024: persist into chunk 0, where the
        # second halves of the k/v projections run as deferred PE filler
        # (phase 1 is DMA-bandwidth-bound; chunks 0-1 are compute-bound
        # with an idle DMA, so the later-needed halves load there).
        # All staged X/W tensors are single [128, (dc, cols)] dc-major
        # tiles filled by one rearranged-AP DMA each.
        vbpool = ctx0.enter_context(tc.tile_pool(name="vb", bufs=1))
        vwpool = ctx0.enter_context(tc.tile_pool(name="vw", bufs=1))
        kbpool = ctx0.enter_context(tc.tile_pool(name="kb", bufs=1))
        kwpool = ctx0.enter_context(tc.tile_pool(name="kw", bufs=1))

        def _ld(eng, tile_ap, src_ap, cols):
            eng.dma_start(
                tile_ap.rearrange("p (dc c) -> p dc c", dc=DC),
                src_ap.rearrange("(dc p) c -> p dc c", p=P))

        # the first matmul group needs wq + xq chunk 0 — those ride the two
        # fast rings as single big DMAs. The gpsimd ring carries, in
        # deadline order: the k/v B-halves (consumed by chunk-0 filler
        # tasks), then xq chunk 1 (chunk-0 end) and wo (chunk 1).
        nc.sync.dma_start(
            wqt_all[:].rearrange("p (dc c) -> p dc c", dc=DC),
            wq.ap().rearrange("(dc p) c -> p dc c", p=P))
        load_xq_chunk(0, eng=nc.scalar)
        kxB = kbpool.tile([P, DC * 1024], f16, tag="kxb", name="kxb")
        vxB = vbpool.tile([P, DC * 1024], f16, tag="xb", name="xb")
        _ld(nc.gpsimd, kxB[:], xk.ap()[:, 1024:2048], 1024)
        _ld(nc.gpsimd, vxB[:], xv.ap()[:, 1024:2048], 1024)
        load_xq_chunk(1)
        for p in range(NPAIR):
            for h in range(D // 512):
                nc.gpsimd.dma_start(
                    wot[p][h][:],
                    wo.ap()[p * P:(p + 1) * P, h * 512:(h + 1) * 512])

        # attention psum pools are created after phase 1 releases ps1's
        # banks (PSUM pools are statically allocated; 8 banks total)
        scps = avps = None
        expp = ctx0.enter_context(tc.tile_pool(name="expp", bufs=6))
        avsb = ctx0.enter_context(tc.tile_pool(name="avsb", bufs=14))
        rcp = ctx0.enter_context(tc.tile_pool(name="rcp", bufs=2))
        osb = ctx0.enter_context(tc.tile_pool(name="osb", bufs=2))
        if not causal:
            mpool = ctx0.enter_context(tc.tile_pool(name="mp", bufs=SM + 2))

        def emit_qproj(j, i):
            # q projection for pair i of sq chunk j -> qTc[(j, i)]
            ps = avps.tile([P, 512], f32, tag="av", name="qps")
            for dc in range(DC):
                nc.tensor.matmul(
                    ps[:], wq_sl(dc, i),
                    xqc[j][:, dc * 512:(dc + 1) * 512],
                    start=(dc == 0), stop=(zero_bias and dc == DC - 1))
            if not zero_bias:
                nc.tensor.matmul(ps[:], bq_t[0:1, i * P:(i + 1) * P],
                                 ones_row_t[:], start=False, stop=True)
            t = qpool.tile([P, 512], f16, tag="qTc", name="qTc")
            nc.vector.tensor_copy(t[:], ps[:])
            qTc[(j, i)] = t

        def emit_outproj_si(si):
            # output projection for one 128-row block of sq
            ot = osb.tile([P, D], f32, tag="ot", name="ot")
            for h in range(D // 512):
                ps = avps.tile([P, 512], f32, tag="av", name="ps3")
                for p in range(NPAIR):
                    nc.tensor.matmul(
                        ps[:],
                        ctxt[p][:, si * P:(si + 1) * P],
                        wot[p][h][:],
                        start=(p == 0), stop=(p == NPAIR - 1))
                nc.vector.tensor_copy(ot[:, h * 512:(h + 1) * 512], ps[:])
            nc.sync.dma_start(out.ap()[si * P:(si + 1) * P, :], ot[:])

        # ---------------- phase 1: k/v projections (+ q chunk 0) ---------
        kwt_all = kwpool.tile([P, DC * GD], f16, tag="kw", name="kw")
        vwt_all = vwpool.tile([P, DC * GD], f16, tag="vw", name="vw")
        with ExitStack() as ctx1:
            xpool = ctx1.enter_context(tc.tile_pool(name="xt", bufs=1))
            xapool = ctx1.enter_context(tc.tile_pool(name="xa", bufs=1))
            ps1 = ctx1.enter_context(tc.tile_pool(name="ps1", bufs=3, space="PSUM"))

            # k on the sync ring, v on the scalar ring — only borrowed this
            # early in phase 1; idle again well before the first exp lands
            # on ACT
            _ld(nc.sync, kwt_all[:], wk.ap(), GD)
            kxA = xpool.tile([P, DC * 1024], f16, tag="xt", name="xt")
            _ld(nc.sync, kxA[:], xk.ap()[:, 0:1024], 1024)
            _ld(nc.scalar, vwt_all[:], wv.ap(), GD)
            vxA = xapool.tile([P, DC * 1024], f16, tag="xa", name="xa")
            _ld(nc.scalar, vxA[:], xv.ap()[:, 0:1024], 1024)

            # q chunk 0 first: unblocks chunk-0 scores as soon as k pair 0
            # lands
            for i in range(NPAIR):
                ps = ps1.tile([P, 512], f32, tag="ps1", name="ps1")
                for dc in range(DC):
                    nc.tensor.matmul(
                        ps[:], wq_sl(dc, i),
                        xqc[0][:, dc * 512:(dc + 1) * 512],
                        start=(dc == 0), stop=(zero_bias and dc == DC - 1))
                if not zero_bias:
                    nc.tensor.matmul(ps[:], bq_t[0:1, i * P:(i + 1) * P],
                                     ones_row_t[:], start=False, stop=True)
                t = qpool.tile([P, 512], f16, tag="qTc", name="qTc")
                nc.vector.tensor_copy(t[:], ps[:])
                qTc[(0, i)] = t

            # k projection for sj 0-1 only (sj 2-3 are deferred into the
            # chunk 0/1 task queues; chunk j only reads kT columns < 512j+512)
            for sj in range(2):
                kx = kxA
                s0 = (sj % 2) * 512
                for i in range(NPAIR):
                    ps = ps1.tile([P, 512], f32, tag="ps1", name="ps1")
                    for dc in range(DC):
                        nc.tensor.matmul(
                            ps[:],
                            kwt_all[:, dc * GD + i * P:dc * GD + (i + 1) * P],
                            kx[:, dc * 1024 + s0:dc * 1024 + s0 + 512],
                            start=(dc == 0),
                            stop=(zero_bias and dc == DC - 1))
                    if not zero_bias:
                        nc.tensor.matmul(
                            ps[:], bk_t[0:1, i * P:(i + 1) * P],
                            ones_row_t[:], start=False, stop=True)
                    nc.vector.tensor_copy(
                        kTt[i][:, sj * 512:(sj + 1) * 512], ps[:])

            # v projection (first half; si 8-15 deferred into chunk 0),
            # natural layout [s, dk] with interleaved ones col
            for si in range(SM // 2):
                ps = ps1.tile([P, 512], f32, tag="ps1", name="ps1")
                for dc in range(DC):
                    nc.tensor.matmul(
                        ps[:],
                        vxA[:, dc * 1024 + si * P:dc * 1024 + (si + 1) * P],
                        vwt_all[:, dc * GD:(dc + 1) * GD],
                        start=(dc == 0),
                        stop=(zero_bias and dc == DC - 1))
                if not zero_bias:
                    nc.tensor.matmul(ps[:], ones_col_t[:], bv_t[:],
                                     start=False, stop=True)
                v3 = vt[si][:].rearrange("p (h c) -> p h c", h=8)
                nc.vector.tensor_copy(
                    v3[:, :, 0:64],
                    ps[:].rearrange("p (h c) -> p h c", h=8))
                nc.vector.memset(v3[:, :, 64:65], 1.0)

        # ---------------- phase 2: attention with deferred fillers -------
        scps = ctx0.enter_context(tc.tile_pool(name="scps", bufs=2, space="PSUM"))
        avps = ctx0.enter_context(tc.tile_pool(name="avps", bufs=3, space="PSUM"))
        fill = []

        def pump():
            while fill:
                t = fill.pop(0)
                if t is None:
                    return  # delay slot
                t()
                return

        def mk_norm(p, e, jj, asb, rv):
            def t():
                r = 2 * p + e
                bc = avps.tile([65, 512], f32, tag="av", name="bc")
                nc.tensor.matmul(bc[0:64, :],
                                 selh_t[:, 64 * r:64 * r + 64],
                                 rv[:], start=True, stop=True)
                nc.vector.tensor_mul(
                    ctxt[p][64 * e:64 * e + 64, jj * 512:(jj + 1) * 512],
                    asb[0:64, :], bc[0:64, :])
            return t

        def mk_vproj(si):
            def t():
                s0 = (si - SM // 2) * P
                ps = avps.tile([P, 512], f32, tag="av", name="vps")
                for dc in range(DC):
                    nc.tensor.matmul(
                        ps[:],
                        vxB[:, dc * 1024 + s0:dc * 1024 + s0 + P],
                        vwt_all[:, dc * GD:(dc + 1) * GD],
                        start=(dc == 0),
                        stop=(zero_bias and dc == DC - 1))
                if not zero_bias:
                    nc.tensor.matmul(ps[:], ones_col_t[:], bv_t[:],
                                     start=False, stop=True)
                v3 = vt[si][:].rearrange("p (h c) -> p h c", h=8)
                nc.vector.tensor_copy(
                    v3[:, :, 0:64],
                    ps[:].rearrange("p (h c) -> p h c", h=8))
                nc.vector.memset(v3[:, :, 64:65], 1.0)
            return t

        def mk_kproj(sj, i):
            def t():
                s0 = (sj - 2) * 512
                ps = avps.tile([P, 512], f32, tag="av", name="kps")
                for dc in range(DC):
                    nc.tensor.matmul(
                        ps[:],
                        kwt_all[:, dc * GD + i * P:dc * GD + (i + 1) * P],
                        kxB[:, dc * 1024 + s0:dc * 1024 + s0 + 512],
                        start=(dc == 0),
                        stop=(zero_bias and dc == DC - 1))
                if not zero_bias:
                    nc.tensor.matmul(
                        ps[:], bk_t[0:1, i * P:(i + 1) * P],
                        ones_row_t[:], start=False, stop=True)
                nc.vector.tensor_copy(
                    kTt[i][:, sj * 512:(sj + 1) * 512], ps[:])
            return t

        # second halves of the k/v projections fill chunk 0-1 PE slack
        # (nothing else is deferred yet there); k sj=2 and v si 8-11 are
        # first needed by chunk 2's m-loop, k sj=3 by chunk 3's
        fill.extend(mk_kproj(2, i) for i in range(NPAIR))
        fill.extend(mk_vproj(si) for si in range(SM // 2, SM))
        fill.extend(mk_kproj(3, i) for i in range(NPAIR))

        for j in range(SJ):
            n_m = 4 * (j + 1) if causal else SM
            # stage the x columns for the q projection two chunks ahead
            load_xq_chunk(j + 2)
            if not causal:
                mt = []
                for m in range(SM):
                    t = mpool.tile([P, 512], bf16, tag="mt", name="mt")
                    nc.sync.dma_start(
                        t[:], maskt.ap()[m * P:(m + 1) * P,
                                         j * 512:(j + 1) * 512])
                    mt.append(t)
            den = avps.tile([8, 512], f32, tag="den", name="den", bufs=1)
            asb_all = {}
            for p in range(NPAIR):
                av = [avps.tile([65, 512], f32, tag="av", name="av")
                      for _ in range(2)]
                pend = []  # (m, exp_tile, c0) awaiting their AV matmuls
                for m in range(n_m):
                    pump()
                    # causal diagonal block d: columns [0, 128d) of this
                    # sq chunk are fully masked -> compute only the
                    # suffix [c0, 512) in scores/exp/AV; the mixed
                    # 128-col window gets the shared p>c mask block
                    d = m - 4 * j if causal else -1
                    c0 = 128 * d if d > 0 else 0
                    nv = 512 - c0
                    sc = scps.tile([P, 1024], f32, tag="sc", name="sc")
                    for e in range(2):
                        nc.tensor.matmul(
                            sc[:, e * 512 + c0:(e + 1) * 512],
                            kTt[p][64 * e:64 * e + 64, m * P:(m + 1) * P],
                            qTc[(j, p)][64 * e:64 * e + 64, c0:512],
                            start=True, stop=True)
                    sc3 = sc[:].rearrange("p (e c) -> p e c", e=2)
                    if causal:
                        if d >= 0:
                            nc.vector.tensor_add(
                                sc3[:, :, c0:c0 + P], sc3[:, :, c0:c0 + P],
                                mask128_t[:][:, None, :].broadcast_to(
                                    [P, 2, P]))
                    else:
                        nc.vector.tensor_add(
                            sc3, sc3,
                            mt[m][:][:, None, :].broadcast_to([P, 2, 512]))
                    ex = expp.tile([P, 1024], f16, tag="ex", name="ex")
                    ex3 = ex[:].rearrange("p (e c) -> p e c", e=2)
                    nc.scalar.activation(ex3[:, :, c0:512],
                                         sc3[:, :, c0:512], Exp, scale=0.125)
                    pend.append((m, ex, c0))
                    if len(pend) > 3:
                        pm, pex, pc0 = pend.pop(0)
                        for e in range(2):
                            nc.tensor.matmul(
                                av[e][:, pc0:512],
                                vt[pm][:, 65 * (2 * p + e):65 * (2 * p + e) + 65],
                                pex[:, e * 512 + pc0:(e + 1) * 512],
                                start=(pm == 0), stop=(pm == n_m - 1))
                for pm, pex, pc0 in pend:
                    # the tail exps lag the PE by a few ACT latencies —
                    # pumped filler keeps the PE busy while they drain
                    pump()
                    for e in range(2):
                        nc.tensor.matmul(
                            av[e][:, pc0:512],
                            vt[pm][:, 65 * (2 * p + e):65 * (2 * p + e) + 65],
                            pex[:, e * 512 + pc0:(e + 1) * 512],
                            start=(pm == 0), stop=(pm == n_m - 1))
                pump()
                # stage av in SBUF; route its denominator row (base
                # partition 64, which matmul rhs allows) into partition
                # 2p+e of the shared den psum tile via a one-hot K=1 MM
                for e in range(2):
                    r = 2 * p + e
                    asb = avsb.tile([65, 512], f16, tag="asb", name="asb")
                    nc.vector.tensor_copy(asb[:], av[e][:])
                    nc.tensor.matmul(den[:], oneh8_t[64:65, 8 * r:8 * r + 8],
                                     asb[64:65, :],
                                     start=(r == 0), stop=(r == 7))
                    asb_all[(p, e)] = asb

            # the batched reciprocal launches on DVE right away (it has no
            # PE dependents until the deferred normalize tasks run), and the
            # NEXT chunk's q projection runs here as boundary filler — the
            # av psum slots are free between chunks, so it doesn't contend
            # with the m-loop's rotating slot
            rv = rcp.tile([8, 512], f16, tag="rinv", name="rinv")
            with nc.allow_low_precision(
                    reason="softmax denominators are O(1..3e4); fp16 "
                           "reciprocal keeps ~5e-4 rel err"):
                nc.vector.reciprocal(rv[:], den[:])
            if j + 1 < SJ:
                for i in range(NPAIR):
                    emit_qproj(j + 1, i)
            # normalize + outproj are deferred into the next chunk's m-loop,
            # behind delay slots so the PE's in-order stream never reaches a
            # reciprocal-dependent instruction early
            nxt = [None] * 2
            for p in range(NPAIR):
                nxt.append(mk_norm(p, 0, j, asb_all[(p, 0)], rv))
                nxt.append(mk_norm(p, 1, j, asb_all[(p, 1)], rv))
            for si in range(4 * j, 4 * j + 4):
                nxt.append(lambda s=si: emit_outproj_si(s))
            fill.extend(nxt)
        while fill:
            pump()

    nc.compile()
    return nc


def _get_nc(s_len, causal, zero_bias):
    key = (s_len, causal, zero_bias)
    if key not in _BUILD_CACHE:
        _BUILD_CACHE[key] = _build(s_len, causal, zero_bias)
    return _BUILD_CACHE[key]


def kernel(query, key, value, mask, Wq, bq, Wk, bk, Wv, bv, Wo, bo):
    import ml_dtypes
    from concourse.bass_utils import run_bass_kernel_spmd

    query = np.asarray(query, dtype=np.float32)
    key = np.asarray(key, dtype=np.float32)
    value = np.asarray(value, dtype=np.float32)
    mask = np.asarray(mask, dtype=np.float32)
    Wq, Wk, Wv, Wo = (np.asarray(w, dtype=np.float32) for w in (Wq, Wk, Wv, Wo))
    bq, bk, bv, bo = (np.asarray(b, dtype=np.float32) for b in (bq, bk, bv, bo))

    b_sz, s_len, d = query.shape
    m2 = mask.reshape(s_len, s_len)
    causal = bool(
        np.array_equal(m2, np.triu(np.ones((s_len, s_len), np.float32), k=1)))

    zero_bias = not (bq.any() or bk.any() or bv.any())
    nc = _get_nc(s_len, causal, zero_bias)

    f16 = np.float16
    ones_row = np.ones((1, 512), f16)
    ones_col = np.ones((1, P), f16)
    ones_vcol = np.ones((P, 8), f16)
    selh = np.zeros((8, 8 * 64), f16)
    for r in range(8):
        selh[r, 64 * r:64 * r + 64] = 1.0
    oneh8 = np.zeros((1, 64), f16)
    oneh8[0, 9 * np.arange(8)] = 1.0
    if causal:
        # maskd[d][p, c] = MASK_NEG where p + 128*d > c
        pp = np.arange(P)[:, None]
        cc = np.arange(512)[None, :]
        maskd = np.concatenate(
            [np.where(pp + P * dd > cc, MASK_NEG, 0.0) for dd in range(4)],
            axis=0).astype(ml_dtypes.bfloat16)
    else:
        maskt = (m2.T * MASK_NEG).astype(ml_dtypes.bfloat16)

    in_maps = []
    for c in range(N_CORES):
        b = c // 2
        g = c % 2
        cols = slice(GD * g, GD * g + GD)
        im = {
            "xq": np.ascontiguousarray(query[b].T).astype(f16),
            "xk": np.ascontiguousarray(key[b].T).astype(f16),
            "xv": np.ascontiguousarray(value[b].T).astype(f16),
            "wq": np.ascontiguousarray(Wq[:, cols]).astype(f16),
            "wk": np.ascontiguousarray(Wk[:, cols]).astype(f16),
            "wv": np.ascontiguousarray(Wv[:, cols]).astype(f16),
            "wo": np.ascontiguousarray(Wo[cols, :]).astype(f16),
            "bq": bq[cols].reshape(1, GD).astype(f16),
            "bk": bk[cols].reshape(1, GD).astype(f16),
            "bv": bv[cols].reshape(1, GD).astype(f16),
            "ones_row": ones_row,
            "ones_col": ones_col,
            "ones_vcol": ones_vcol,
            "selh": selh,
            "oneh8": oneh8,
        }
        if causal:
            im["maskd"] = maskd
        else:
            im["maskt"] = maskt
        in_maps.append(im)

    res = run_bass_kernel_spmd(nc, in_maps, list(range(N_CORES)))

    out = np.empty((b_sz, s_len, d), np.float32)
    for b in range(b_sz):
        out[b] = res.results[2 * b]["out"] + res.results[2 * b + 1]["out"] + bo
    return out


# revision 39
# speedup vs baseline: 1.1867x; 1.0393x over previous
"""Multi-head causal attention (B=4, S=2048, D=1024, H=16) on 8 Trainium2
NeuronCores via Bass/Tile.

Sharding: core c handles batch b = c//2 and head-group g = c%2 (8 heads,
i.e. columns [512g, 512g+512) of Wq/Wk/Wv and rows [512g, 512g+512) of Wo).
Each core computes its 8 heads' attention and a partial output projection
[S, D]; the host sums the two head-group partials per batch and adds bo.

Matmul operands are fp16 (full-rate 1 cycle/row on the PE; fp32 accumulate
in PSUM); softmax runs in fp32 on ACT/DVE. All values are O(100) or less so
fp16 range is safe, and fp16's 10-bit mantissa keeps the end-to-end error
around 5e-4. Layouts keep every matmul at N=512 moving columns:
  qT/kT:  [dk, s]  (projection emitted transposed: lhsT=W chunk, rhs=X^T)
  v:      [s, dk]  interleaved with a ones column per head ([..v_h.., 1])
          so the attention-V matmul also produces the softmax row-sums
  scores: [sk, sq] (transposed; lhsT=kT chunk, rhs=qT) -> exp -> expT
  AV:     av[65, sq] += v_aug^T @ expT  (row 64 = softmax denominators)
  out:    partial[sq, :] = ctx^T.T @ Wo  (ctx^T is exactly the AV output)

Scheduling: the attention m-loop is ACT(exp)-rate-limited, so all serial
chunk-boundary work (reciprocal, denominator broadcast, ctx normalize,
output projection, and the NEXT chunk's q-projection) is deferred into a
task queue pumped one task per (pair, m) iteration — the PE never idles at
chunk boundaries and stays at full p-state.
"""

import os
import sys
import numpy as np

for _p in ("/opt/trn_rl_repo", "/root/.axon_site/_ro/trn_rl_repo"):
    if _p not in sys.path:
        sys.path.append(_p)

B, S_FULL, D, H, DK = 4, 2048, 1024, 16, 64
GD = 512          # dk span per core (8 heads)
P = 128
NPAIR = GD // P   # 4 head-pairs per core
N_CORES = 8
MASK_NEG = -8.0e9  # multiplied by the 0.125 softmax scale inside exp -> -1e9

_BUILD_CACHE = {}


def _build(s_len, causal, zero_bias):
    from contextlib import ExitStack

    import concourse.tile as tile
    from concourse import bacc, mybir

    dt = mybir.dt
    f32, f16, bf16 = dt.float32, dt.float16, dt.bfloat16
    Exp = mybir.ActivationFunctionType.Exp

    S = s_len
    SJ = S // 512     # 512-wide sq chunks
    SM = S // P       # 128-wide sk chunks
    DC = D // P       # contraction chunks for the projections

    nc = bacc.Bacc("TRN2", target_bir_lowering=False, debug=False,
                   num_devices=N_CORES)

    xq = nc.dram_tensor("xq", [D, S], f16, kind="ExternalInput")
    xk = nc.dram_tensor("xk", [D, S], f16, kind="ExternalInput")
    xv = nc.dram_tensor("xv", [D, S], f16, kind="ExternalInput")
    wq = nc.dram_tensor("wq", [D, GD], f16, kind="ExternalInput")
    wk = nc.dram_tensor("wk", [D, GD], f16, kind="ExternalInput")
    wv = nc.dram_tensor("wv", [D, GD], f16, kind="ExternalInput")
    wo = nc.dram_tensor("wo", [GD, D], f16, kind="ExternalInput")
    bq = nc.dram_tensor("bq", [1, GD], f16, kind="ExternalInput")
    bk = nc.dram_tensor("bk", [1, GD], f16, kind="ExternalInput")
    bv = nc.dram_tensor("bv", [1, GD], f16, kind="ExternalInput")
    ones_row = nc.dram_tensor("ones_row", [1, 512], f16, kind="ExternalInput")
    ones_col = nc.dram_tensor("ones_col", [1, P], f16, kind="ExternalInput")
    ones_vcol = nc.dram_tensor("ones_vcol", [P, 8], f16, kind="ExternalInput")
    # selh[:, 64r:64r+64] is row-r-one-hot: selects head r's reciprocal row
    # and broadcasts it over 64 partitions in one K=8 matmul
    selh = nc.dram_tensor("selh", [8, 8 * 64], f16, kind="ExternalInput")
    # oneh8[0, 8r:8r+8] is the one-hot row e_r: routes head r's softmax
    # denominator row into partition r of the gathered [8, 512] psum tile
    oneh8 = nc.dram_tensor("oneh8", [1, 64], f16, kind="ExternalInput")
    if causal:
        # 4 canonical diagonal-band blocks: block d, entry [p, c] masked
        # when p + 128*d > c  (value MASK_NEG, else 0)
        maskd = nc.dram_tensor("maskd", [4 * P, 512], bf16, kind="ExternalInput")
    else:
        # full transposed mask [sk, sq] * MASK_NEG
        maskt = nc.dram_tensor("maskt", [S, S], bf16, kind="ExternalInput")
    out = nc.dram_tensor("out", [S, D], f32, kind="ExternalOutput")

    with tile.TileContext(nc) as tc, ExitStack() as ctx0:
        persist = ctx0.enter_context(tc.tile_pool(name="persist", bufs=1))

        kTt = [persist.tile([P, S], f16, tag=f"kT{p}", name=f"kT{p}")
               for p in range(NPAIR)]
        vt = [persist.tile([P, 8 * 65], f16, tag=f"v{m}", name=f"v{m}")
              for m in range(SM)]
        ctxt = [persist.tile([P, S], f16, tag=f"ctx{p}", name=f"ctx{p}")
                for p in range(NPAIR)]
        # wq persists: the q projection for sq chunk j+1 is emitted as
        # filler inside chunk j's attention loop. One [128, DC*512] tile,
        # dc-major, filled by a single rearranged-AP DMA
        wqt_all = persist.tile([P, DC * GD], f16, tag="wq", name="wq")

        def wq_sl(dc, i):
            return wqt_all[:, dc * GD + i * P:dc * GD + (i + 1) * P]
        wot = [[persist.tile([P, 512], f16, tag=f"wo{p}_{h}", name=f"wo{p}_{h}")
                for h in range(D // 512)] for p in range(NPAIR)]
        if not zero_bias:
            ones_row_t = persist.tile([1, 512], f16, tag="ones_row")
            ones_col_t = persist.tile([1, P], f16, tag="ones_col")
            bq_t = persist.tile([1, GD], f16, tag="bq")
            bk_t = persist.tile([1, GD], f16, tag="bk")
            bv_t = persist.tile([1, GD], f16, tag="bv")
            nc.gpsimd.dma_start(ones_row_t[:], ones_row.ap())
            nc.gpsimd.dma_start(ones_col_t[:], ones_col.ap())
            nc.gpsimd.dma_start(bq_t[:], bq.ap())
            nc.gpsimd.dma_start(bk_t[:], bk.ap())
            nc.gpsimd.dma_start(bv_t[:], bv.ap())
        else:
            ones_row_t = ones_col_t = bq_t = bk_t = bv_t = None
        selh_t = persist.tile([8, 8 * 64], f16, tag="selh")
        # staged at partition 64 so its base matches asb[64:65] in the
        # denominator-gather matmul (lhsT/rhs bases must agree)
        oneh8_t = persist.tile([65, 64], f16, tag="oneh8")
        nc.gpsimd.dma_start(selh_t[:], selh.ap())
        nc.gpsimd.dma_start(oneh8_t[64:65, :], oneh8.ap())
        if causal:
            # one [128,128] block covers every diagonal mixed window:
            # within the window the pattern is always "masked iff p > c"
            mask128_t = persist.tile([P, P], bf16, tag="mask128")
            nc.gpsimd.dma_start(mask128_t[:], maskd.ap()[0:P, 0:P])

        # q-projection staging: per-chunk X^T column slices and per-chunk
        # qT output tiles (chunk j's qT is produced during chunk j-1)
        qpool = ctx0.enter_context(tc.tile_pool(name="qp", bufs=8))
        xqpool = ctx0.enter_context(tc.tile_pool(name="xqp", bufs=2))
        xqc = {}
        qTc = {}

        def load_xq_chunk(j, eng=None):
            # one DMA per chunk: dst is [128, (dc, 512)] dc-major, src AP
            # rearranged so row dc*128+p lands on partition p
            if j >= SJ:
                return
            t = xqpool.tile([P, DC * 512], f16, tag="xqc", name="xqc")
            (eng or nc.gpsimd).dma_start(
                t[:].rearrange("p (dc c) -> p dc c", dc=DC),
                xq.ap()[:, j * 512:(j + 1) * 512].rearrange(
                    "(dc p) c -> p dc c", p=P))
            xqc[j] = t

        # X/W staging: phase 1 is DMA-bandwidth-bound, so only the
        # earliest-needed slices (wq, xq0, wk, xk sj0) are consumed there;
        # every remaining projection runs as an arrival-ordered filler task
        # inside the attention chunks, with its input DMA scheduled by
        # deadline. All staged tensors are [128, (dc, cols)] dc-major
        # tiles filled by one rearranged-AP DMA each.
        vwpool = ctx0.enter_context(tc.tile_pool(name="vw", bufs=1))
        kwpool = ctx0.enter_context(tc.tile_pool(name="kw", bufs=1))
        kq2pool = ctx0.enter_context(tc.tile_pool(name="kq2", bufs=1))
        vq1pool = ctx0.enter_context(tc.tile_pool(name="vq1", bufs=1))
        vq2pool = ctx0.enter_context(tc.tile_pool(name="vq2", bufs=1))
        kbpool = ctx0.enter_context(tc.tile_pool(name="kb", bufs=1))
        vbpool = ctx0.enter_context(tc.tile_pool(name="vb", bufs=1))

        def _ld(eng, tile_ap, src_ap):
            eng.dma_start(
                tile_ap.rearrange("p (dc c) -> p dc c", dc=DC),
                src_ap.rearrange("(dc p) c -> p dc c", p=P))

        kwt_all = kwpool.tile([P, DC * GD], f16, tag="kw", name="kw")
        vwt_all = vwpool.tile([P, DC * GD], f16, tag="vw", name="vw")
        kxq2 = kq2pool.tile([P, DC * 512], f16, tag="kq2", name="kq2")
        vxq1 = vq1pool.tile([P, DC * 512], f16, tag="vq1", name="vq1")
        vxq2 = vq2pool.tile([P, DC * 512], f16, tag="vq2", name="vq2")
        kxB = kbpool.tile([P, DC * 1024], f16, tag="kxb", name="kxb")
        vxB = vbpool.tile([P, DC * 1024], f16, tag="xb", name="xb")

        # sync ring: q weights, then k's phase-1 needs, then k sj1
        nc.sync.dma_start(
            wqt_all[:].rearrange("p (dc c) -> p dc c", dc=DC),
            wq.ap().rearrange("(dc p) c -> p dc c", p=P))
        # scalar ring: xq chunk 0, then v's chunk-0 needs
        load_xq_chunk(0, eng=nc.scalar)
        _ld(nc.scalar, vwt_all[:], wv.ap())
        _ld(nc.scalar, vxq1[:], xv.ap()[:, 0:512])
        _ld(nc.scalar, vxq2[:], xv.ap()[:, 512:1024])
        # gpsimd ring: chunk 1+ inputs in deadline order
        _ld(nc.gpsimd, kxB[:], xk.ap()[:, 1024:2048])
        _ld(nc.gpsimd, vxB[:], xv.ap()[:, 1024:2048])
        load_xq_chunk(1)
        for p in range(NPAIR):
            for h in range(D // 512):
                nc.gpsimd.dma_start(
                    wot[p][h][:],
                    wo.ap()[p * P:(p + 1) * P, h * 512:(h + 1) * 512])

        # attention psum pools are created after phase 1 releases ps1's
        # banks (PSUM pools are statically allocated; 8 banks total)
        scps = avps = None
        expp = ctx0.enter_context(tc.tile_pool(name="expp", bufs=6))
        avsb = ctx0.enter_context(tc.tile_pool(name="avsb", bufs=14))
        rcp = ctx0.enter_context(tc.tile_pool(name="rcp", bufs=2))
        osb = ctx0.enter_context(tc.tile_pool(name="osb", bufs=2))
        if not causal:
            mpool = ctx0.enter_context(tc.tile_pool(name="mp", bufs=SM + 2))

        def emit_qproj(j, i):
            # q projection for pair i of sq chunk j -> qTc[(j, i)]
            ps = avps.tile([P, 512], f32, tag="av", name="qps")
            for dc in range(DC):
                nc.tensor.matmul(
                    ps[:], wq_sl(dc, i),
                    xqc[j][:, dc * 512:(dc + 1) * 512],
                    start=(dc == 0), stop=(zero_bias and dc == DC - 1))
            if not zero_bias:
                nc.tensor.matmul(ps[:], bq_t[0:1, i * P:(i + 1) * P],
                                 ones_row_t[:], start=False, stop=True)
            t = qpool.tile([P, 512], f16, tag="qTc", name="qTc")
            nc.vector.tensor_copy(t[:], ps[:])
            qTc[(j, i)] = t

        def emit_outproj_si(si):
            # output projection for one 128-row block of sq
            ot = osb.tile([P, D], f32, tag="ot", name="ot")
            for h in range(D // 512):
                ps = avps.tile([P, 512], f32, tag="av", name="ps3")
                for p in range(NPAIR):
                    nc.tensor.matmul(
                        ps[:],
                        ctxt[p][:, si * P:(si + 1) * P],
                        wot[p][h][:],
                        start=(p == 0), stop=(p == NPAIR - 1))
                nc.vector.tensor_copy(ot[:, h * 512:(h + 1) * 512], ps[:])
            nc.sync.dma_start(out.ap()[si * P:(si + 1) * P, :], ot[:])

        # ---------------- phase 1: q chunk 0 + k sj 0 --------------------
        with ExitStack() as ctx1:
            xpool = ctx1.enter_context(tc.tile_pool(name="xt", bufs=1))
            ps1 = ctx1.enter_context(tc.tile_pool(name="ps1", bufs=3, space="PSUM"))

            # the two k slices phase 1 actually consumes trail wq on sync
            _ld(nc.sync, kwt_all[:], wk.ap())
            kxq1 = xpool.tile([P, DC * 512], f16, tag="xt", name="xt")
            _ld(nc.sync, kxq1[:], xk.ap()[:, 0:512])
            _ld(nc.sync, kxq2[:], xk.ap()[:, 512:1024])

            # q chunk 0 first: unblocks chunk-0 scores as soon as k pair 0
            # lands
            for i in range(NPAIR):
                ps = ps1.tile([P, 512], f32, tag="ps1", name="ps1")
                for dc in range(DC):
                    nc.tensor.matmul(
                        ps[:], wq_sl(dc, i),
                        xqc[0][:, dc * 512:(dc + 1) * 512],
                        start=(dc == 0), stop=(zero_bias and dc == DC - 1))
                if not zero_bias:
                    nc.tensor.matmul(ps[:], bq_t[0:1, i * P:(i + 1) * P],
                                     ones_row_t[:], start=False, stop=True)
                t = qpool.tile([P, 512], f16, tag="qTc", name="qTc")
                nc.vector.tensor_copy(t[:], ps[:])
                qTc[(0, i)] = t

            # k projection for sj 0 only (sj 1-3 are deferred into the
            # chunk task queues; chunk j only reads kT columns < 512j+512)
            for i in range(NPAIR):
                ps = ps1.tile([P, 512], f32, tag="ps1", name="ps1")
                for dc in range(DC):
                    nc.tensor.matmul(
                        ps[:],
                        kwt_all[:, dc * GD + i * P:dc * GD + (i + 1) * P],
                        kxq1[:, dc * 512:(dc + 1) * 512],
                        start=(dc == 0),
                        stop=(zero_bias and dc == DC - 1))
                if not zero_bias:
                    nc.tensor.matmul(
                        ps[:], bk_t[0:1, i * P:(i + 1) * P],
                        ones_row_t[:], start=False, stop=True)
                nc.vector.tensor_copy(
                    kTt[i][:, 0:512], ps[:])

        # ---------------- phase 2: attention with deferred fillers -------
        scps = ctx0.enter_context(tc.tile_pool(name="scps", bufs=2, space="PSUM"))
        avps = ctx0.enter_context(tc.tile_pool(name="avps", bufs=3, space="PSUM"))
        fill = []

        def pump():
            while fill:
                t = fill.pop(0)
                if t is None:
                    return  # delay slot
                t()
                return

        def mk_norm(p, e, jj, asb, rv):
            def t():
                r = 2 * p + e
                bc = avps.tile([65, 512], f32, tag="av", name="bc")
                nc.tensor.matmul(bc[0:64, :],
                                 selh_t[:, 64 * r:64 * r + 64],
                                 rv[:], start=True, stop=True)
                nc.vector.tensor_mul(
                    ctxt[p][64 * e:64 * e + 64, jj * 512:(jj + 1) * 512],
                    asb[0:64, :], bc[0:64, :])
            return t

        def mk_vproj(si):
            def t():
                if si < 4:
                    src, s0 = vxq1, si * P
                elif si < 8:
                    src, s0 = vxq2, (si - 4) * P
                else:
                    src, s0 = vxB, (si - 8) * P
                w = 512 if si < 8 else 1024
                ps = avps.tile([P, 512], f32, tag="av", name="vps")
                for dc in range(DC):
                    nc.tensor.matmul(
                        ps[:],
                        src[:, dc * w + s0:dc * w + s0 + P],
                        vwt_all[:, dc * GD:(dc + 1) * GD],
                        start=(dc == 0),
                        stop=(zero_bias and dc == DC - 1))
                if not zero_bias:
                    nc.tensor.matmul(ps[:], ones_col_t[:], bv_t[:],
                                     start=False, stop=True)
                v3 = vt[si][:].rearrange("p (h c) -> p h c", h=8)
                nc.vector.tensor_copy(
                    v3[:, :, 0:64],
                    ps[:].rearrange("p (h c) -> p h c", h=8))
                nc.vector.memset(v3[:, :, 64:65], 1.0)
            return t

        def mk_kproj(sj, i):
            def t():
                if sj == 1:
                    src, s0, w = kxq2, 0, 512
                else:
                    src, s0, w = kxB, (sj - 2) * 512, 1024
                ps = avps.tile([P, 512], f32, tag="av", name="kps")
                for dc in range(DC):
                    nc.tensor.matmul(
                        ps[:],
                        kwt_all[:, dc * GD + i * P:dc * GD + (i + 1) * P],
                        src[:, dc * w + s0:dc * w + s0 + 512],
                        start=(dc == 0),
                        stop=(zero_bias and dc == DC - 1))
                if not zero_bias:
                    nc.tensor.matmul(
                        ps[:], bk_t[0:1, i * P:(i + 1) * P],
                        ones_row_t[:], start=False, stop=True)
                nc.vector.tensor_copy(
                    kTt[i][:, sj * 512:(sj + 1) * 512], ps[:])
            return t

        # all remaining k/v projections fill chunk 0-1 PE slack, ordered by
        # deadline: v si 0-3 feed chunk 0's own AV, k sj=1 and v si 4-7
        # feed chunk 1, k sj=2 / v si 8-11 chunk 2, k sj=3 / v 12-15
        # chunk 3
        fill.extend(mk_vproj(si) for si in range(4))
        fill.extend(mk_kproj(1, i) for i in range(NPAIR))
        fill.extend(mk_vproj(si) for si in range(4, 8))
        fill.extend(mk_kproj(2, i) for i in range(NPAIR))
        fill.extend(mk_vproj(si) for si in range(8, 12))
        fill.extend(mk_kproj(3, i) for i in range(NPAIR))
        fill.extend(mk_vproj(si) for si in range(12, 16))

        for j in range(SJ):
            n_m = 4 * (j + 1) if causal else SM
            # stage the x columns for the q projection two chunks ahead
            load_xq_chunk(j + 2)
            if not causal:
                mt = []
                for m in range(SM):
                    t = mpool.tile([P, 512], bf16, tag="mt", name="mt")
                    nc.sync.dma_start(
                        t[:], maskt.ap()[m * P:(m + 1) * P,
                                         j * 512:(j + 1) * 512])
                    mt.append(t)
            den = avps.tile([8, 512], f32, tag="den", name="den", bufs=1)
            asb_all = {}
            for p in range(NPAIR):
                av = [avps.tile([65, 512], f32, tag="av", name="av")
                      for _ in range(2)]
                pend = []  # (m, exp_tile, c0) awaiting their AV matmuls
                for m in range(n_m):
                    pump()
                    # causal diagonal block d: columns [0, 128d) of this
                    # sq chunk are fully masked -> compute only the
                    # suffix [c0, 512) in scores/exp/AV; the mixed
                    # 128-col window gets the shared p>c mask block
                    d = m - 4 * j if causal else -1
                    c0 = 128 * d if d > 0 else 0
                    nv = 512 - c0
                    sc = scps.tile([P, 1024], f32, tag="sc", name="sc")
                    for e in range(2):
                        nc.tensor.matmul(
                            sc[:, e * 512 + c0:(e + 1) * 512],
                            kTt[p][64 * e:64 * e + 64, m * P:(m + 1) * P],
                            qTc[(j, p)][64 * e:64 * e + 64, c0:512],
                            start=True, stop=True)
                    sc3 = sc[:].rearrange("p (e c) -> p e c", e=2)
                    if causal:
                        if d >= 0:
                            nc.vector.tensor_add(
                                sc3[:, :, c0:c0 + P], sc3[:, :, c0:c0 + P],
                                mask128_t[:][:, None, :].broadcast_to(
                                    [P, 2, P]))
                    else:
                        nc.vector.tensor_add(
                            sc3, sc3,
                            mt[m][:][:, None, :].broadcast_to([P, 2, 512]))
                    ex = expp.tile([P, 1024], f16, tag="ex", name="ex")
                    ex3 = ex[:].rearrange("p (e c) -> p e c", e=2)
                    nc.scalar.activation(ex3[:, :, c0:512],
                                         sc3[:, :, c0:512], Exp, scale=0.125)
                    pend.append((m, ex, c0))
                    if len(pend) > 3:
                        pm, pex, pc0 = pend.pop(0)
                        for e in range(2):
                            nc.tensor.matmul(
                                av[e][:, pc0:512],
                                vt[pm][:, 65 * (2 * p + e):65 * (2 * p + e) + 65],
                                pex[:, e * 512 + pc0:(e + 1) * 512],
                                start=(pm == 0), stop=(pm == n_m - 1))
                for pm, pex, pc0 in pend:
                    # the tail exps lag the PE by a few ACT latencies —
                    # pumped filler keeps the PE busy while they drain
                    pump()
                    for e in range(2):
                        nc.tensor.matmul(
                            av[e][:, pc0:512],
                            vt[pm][:, 65 * (2 * p + e):65 * (2 * p + e) + 65],
                            pex[:, e * 512 + pc0:(e + 1) * 512],
                            start=(pm == 0), stop=(pm == n_m - 1))
                pump()
                # stage av in SBUF; route its denominator row (base
                # partition 64, which matmul rhs allows) into partition
                # 2p+e of the shared den psum tile via a one-hot K=1 MM
                for e in range(2):
                    r = 2 * p + e
                    asb = avsb.tile([65, 512], f16, tag="asb", name="asb")
                    nc.vector.tensor_copy(asb[:], av[e][:])
                    nc.tensor.matmul(den[:], oneh8_t[64:65, 8 * r:8 * r + 8],
                                     asb[64:65, :],
                                     start=(r == 0), stop=(r == 7))
                    asb_all[(p, e)] = asb

            # the batched reciprocal launches on DVE right away (it has no
            # PE dependents until the deferred normalize tasks run), and the
            # NEXT chunk's q projection runs here as boundary filler — the
            # av psum slots are free between chunks, so it doesn't contend
            # with the m-loop's rotating slot
            rv = rcp.tile([8, 512], f16, tag="rinv", name="rinv")
            with nc.allow_low_precision(
                    reason="softmax denominators are O(1..3e4); fp16 "
                           "reciprocal keeps ~5e-4 rel err"):
                nc.vector.reciprocal(rv[:], den[:])
            if j + 1 < SJ:
                for i in range(NPAIR):
                    emit_qproj(j + 1, i)
            # normalize + outproj are deferred into the next chunk's m-loop,
            # behind delay slots so the PE's in-order stream never reaches a
            # reciprocal-dependent instruction early
            nxt = [None] * 2
            for p in range(NPAIR):
                nxt.append(mk_norm(p, 0, j, asb_all[(p, 0)], rv))
                nxt.append(mk_norm(p, 1, j, asb_all[(p, 1)], rv))
            for si in range(4 * j, 4 * j + 4):
                nxt.append(lambda s=si: emit_outproj_si(s))
            fill.extend(nxt)
        while fill:
            pump()

    nc.compile()
    return nc


def _get_nc(s_len, causal, zero_bias):
    key = (s_len, causal, zero_bias)
    if key not in _BUILD_CACHE:
        _BUILD_CACHE[key] = _build(s_len, causal, zero_bias)
    return _BUILD_CACHE[key]


def kernel(query, key, value, mask, Wq, bq, Wk, bk, Wv, bv, Wo, bo):
    import ml_dtypes
    from concourse.bass_utils import run_bass_kernel_spmd

    query = np.asarray(query, dtype=np.float32)
    key = np.asarray(key, dtype=np.float32)
    value = np.asarray(value, dtype=np.float32)
    mask = np.asarray(mask, dtype=np.float32)
    Wq, Wk, Wv, Wo = (np.asarray(w, dtype=np.float32) for w in (Wq, Wk, Wv, Wo))
    bq, bk, bv, bo = (np.asarray(b, dtype=np.float32) for b in (bq, bk, bv, bo))

    b_sz, s_len, d = query.shape
    m2 = mask.reshape(s_len, s_len)
    causal = bool(
        np.array_equal(m2, np.triu(np.ones((s_len, s_len), np.float32), k=1)))

    zero_bias = not (bq.any() or bk.any() or bv.any())
    nc = _get_nc(s_len, causal, zero_bias)

    f16 = np.float16
    ones_row = np.ones((1, 512), f16)
    ones_col = np.ones((1, P), f16)
    ones_vcol = np.ones((P, 8), f16)
    selh = np.zeros((8, 8 * 64), f16)
    for r in range(8):
        selh[r, 64 * r:64 * r + 64] = 1.0
    oneh8 = np.zeros((1, 64), f16)
    oneh8[0, 9 * np.arange(8)] = 1.0
    if causal:
        # maskd[d][p, c] = MASK_NEG where p + 128*d > c
        pp = np.arange(P)[:, None]
        cc = np.arange(512)[None, :]
        maskd = np.concatenate(
            [np.where(pp + P * dd > cc, MASK_NEG, 0.0) for dd in range(4)],
            axis=0).astype(ml_dtypes.bfloat16)
    else:
        maskt = (m2.T * MASK_NEG).astype(ml_dtypes.bfloat16)

    in_maps = []
    for c in range(N_CORES):
        b = c // 2
        g = c % 2
        cols = slice(GD * g, GD * g + GD)
        im = {
            "xq": np.ascontiguousarray(query[b].T).astype(f16),
            "xk": np.ascontiguousarray(key[b].T).astype(f16),
            "xv": np.ascontiguousarray(value[b].T).astype(f16),
            "wq": np.ascontiguousarray(Wq[:, cols]).astype(f16),
            "wk": np.ascontiguousarray(Wk[:, cols]).astype(f16),
            "wv": np.ascontiguousarray(Wv[:, cols]).astype(f16),
            "wo": np.ascontiguousarray(Wo[cols, :]).astype(f16),
            "bq": bq[cols].reshape(1, GD).astype(f16),
            "bk": bk[cols].reshape(1, GD).astype(f16),
            "bv": bv[cols].reshape(1, GD).astype(f16),
            "ones_row": ones_row,
            "ones_col": ones_col,
            "ones_vcol": ones_vcol,
            "selh": selh,
            "oneh8": oneh8,
        }
        if causal:
            im["maskd"] = maskd
        else:
            im["maskt"] = maskt
        in_maps.append(im)

    res = run_bass_kernel_spmd(nc, in_maps, list(range(N_CORES)))

    out = np.empty((b_sz, s_len, d), np.float32)
    for b in range(b_sz):
        out[b] = res.results[2 * b]["out"] + res.results[2 * b + 1]["out"] + bo
    return out
